# revision 4
# baseline (speedup 1.0000x reference)
"""Trainium2 Bass kernel v6 for nn_AffineContour — parity-split AllGather.

Design (8 cores):
- Even cores compute the u-MLP, odd cores the v-MLP; each parity group
  of 4 row-shards its (65536, 512) W0 4 ways (8.39 MB fp8 per core).
- W0 host-quantized to fp8e4 per-column; x_even fp8 with one global
  scale; scales fold into W1 rows (relu(s z) = s relu(z), s>0); W1 bf16.
- GEMV runs in fp8 DoubleRow perf mode: 64 matmuls contracting 256 rows
  each (the existing weight packing is already the DoubleRow interleave).
- ONE ncfw AllGather with two parity replica groups [[0,2,4,6],[1,3,5,7]]
  exchanges the [1,512] bf16 partials inside each parity only (4-rank
  mesh). Summing + transposing the 4 gathered rows is 4 small matmuls.
- The parities never talk on-device: vals = u_s*x_odd + u_t splits into
  a u-only and a v-only term; even core 2p outputs u_s*xo[pair p], odd
  core 2p+1 outputs u_t broadcast, and the HOST adds the two vectors.
"""

import threading

import ml_dtypes
import numpy as np

import concourse.bacc as bacc
import concourse.mybir as mybir
import concourse.tile as tile
from concourse.bass_utils import run_bass_kernel_spmd

V = 131072
VH = 65536
F = 512
NCORES = 8
CHUNKS = 128          # 128-row chunks per core (16384 rows)
DG = 64               # DoubleRow double-groups (2 chunks each)
NB = 8                # weight DMA blocks (1 MB each)
F32 = mybir.dt.float32
BF16 = mybir.dt.bfloat16
FP8 = mybir.dt.float8e4

NP_FP8 = ml_dtypes.float8_e4m3   # IEEE e4m3, max 240 — matches TRN fp8e4
NP_BF16 = ml_dtypes.bfloat16

_lock = threading.Lock()
_cache = {}


def build_nc():
    nc = bacc.Bacc(
        "TRN2",
        debug=False,
        enable_asserts=False,
        target_bir_lowering=False,
        num_devices=NCORES,
    )

    w0b = nc.dram_tensor("w0b", [NB, 128, 8192], FP8, kind="ExternalInput")
    xe8_d = nc.dram_tensor("xe8", [128, CHUNKS], FP8, kind="ExternalInput")
    w1_d = nc.dram_tensor("w1", [128, 4 * F], BF16, kind="ExternalInput")
    b0s_d = nc.dram_tensor("b0s", [128, 4], F32, kind="ExternalInput")
    b1_d = nc.dram_tensor("b1", [1, F], BF16, kind="ExternalInput")
    hw_d = nc.dram_tensor("hw", [1, F], F32, kind="ExternalInput")
    hb_d = nc.dram_tensor("hb", [1, 1], F32, kind="ExternalInput")
    selm_d = nc.dram_tensor("selm", [1, 2], F32, kind="ExternalInput")
    ones4_d = nc.dram_tensor("ones4", [4, 1], BF16, kind="ExternalInput")
    xo_d = nc.dram_tensor("xo_t", [128, 128], F32, kind="ExternalInput")
    vals_out = nc.dram_tensor("vals_out", [VH // 4], F32,
                              kind="ExternalOutput")

    with tile.TileContext(nc) as tc:
        with (
            tc.tile_pool(name="wpool", bufs=6) as wpool,
            tc.tile_pool(name="spool", bufs=1) as spool,
            tc.tile_pool(name="psum", bufs=1, space="PSUM") as psum,
            tc.tile_pool(name="dram", bufs=1, space="DRAM") as dram,
        ):
            xe = spool.tile([128, CHUNKS], FP8)
            nc.sync.dma_start(xe[:], xe8_d[:])
            w1_sb = spool.tile([128, 4 * F], BF16)
            b0s = spool.tile([128, 4], F32)
            nc.gpsimd.dma_start(b0s[:], b0s_d[:])
            b1_sb = spool.tile([1, F], BF16)
            nc.gpsimd.dma_start(b1_sb[:], b1_d[:])
            hw_sb = spool.tile([1, F], F32)
            nc.gpsimd.dma_start(hw_sb[:], hw_d[:])
            hb_sb = spool.tile([1, 1], F32)
            nc.gpsimd.dma_start(hb_sb[:], hb_d[:])
            selm = spool.tile([1, 2], F32)
            nc.gpsimd.dma_start(selm[:], selm_d[:])
            ones4 = spool.tile([4, 1], BF16)
            nc.gpsimd.dma_start(ones4[:], ones4_d[:])
            xo = spool.tile([128, 128], F32)
            nc.gpsimd.dma_start(xo[:], xo_d[:])

            one_b = spool.tile([1, 1], BF16)
            nc.vector.memset(one_b[:], 1.0)
            ones128 = spool.tile([1, 128], F32)
            nc.vector.memset(ones128[:], 1.0)

            # dummy warm-up collective: wakes ncfw/TOPSP during the GEMV so
            # the real AllGather's trigger->begin latency is hot
            wu = spool.tile([1, 16], F32)
            nc.vector.memset(wu[:], 0.0)
            wu_in = dram.tile([1, 16], F32)
            wu_out = dram.tile([4, 16], F32)
            nc.sync.dma_start(wu_in[:], wu[:])
            nc.gpsimd.collective_compute(
                "AllGather",
                mybir.AluOpType.bypass,
                replica_groups=[[0, 2, 4, 6], [1, 3, 5, 7]],
                ins=[wu_in[:].opt()],
                outs=[wu_out[:].opt()],
            )

            # --- layer-1 GEMV: 64 DoubleRow matmuls, K=256 each ---
            # Tile tracks DMA deps per TILE, so block size sets how long
            # the first/last matmul waits: stream 4x128KB head pieces,
            # 7x1MB middle blocks, 4x128KB tail pieces. The 512 KB w1
            # load is deferred behind most of the weight stream.
            dma_engines = [nc.sync, nc.scalar]
            psum1 = psum.tile([1, F], F32, name="psum1")

            def dr_matmul(dg, wt, t):
                # xe pair for dg lives at cols 32b+i and 32b+i+16
                # (LDWEIGHTS DoubleRow needs 16-element pair stride)
                b, i = dg // 16, dg % 16
                nc.tensor.matmul(
                    psum1[:],
                    xe[:, 32 * b : 32 * b + 32].rearrange(
                        "p (k i) -> p k i", k=2)[:, :, i : i + 1],
                    wt[:, 1024 * t : 1024 * (t + 1)].rearrange(
                        "p (k n) -> p k n", k=2),
                    start=(dg == 0),
                    stop=(dg == DG - 1),
                    perf_mode=mybir.MatmulPerfMode.DoubleRow,
                )

            for g in range(NB):
                wt = wpool.tile([128, 8192], FP8, tag="wmid")
                if g == NB - 1:
                    for q in range(4):
                        dma_engines[q % 2].dma_start(
                            wt[:, 2048 * q : 2048 * (q + 1)],
                            w0b.ap()[g][:, 2048 * q : 2048 * (q + 1)],
                        )
                else:
                    dma_engines[g % 2].dma_start(wt[:], w0b.ap()[g])
                if g == 6:
                    nc.scalar.dma_start(w1_sb[:], w1_d[:])
                for t in range(8):
                    dr_matmul(8 * g + t, wt, t)
            partial = spool.tile([1, F], BF16)
            nc.vector.tensor_copy(partial[:], psum1[:])

            # --- AllGather [1,512] bf16 within each parity group of 4 ---
            cc_in = dram.tile([1, F], BF16)
            cc_out = dram.tile([4, F], BF16)
            nc.sync.dma_start(cc_in[:], partial[:])
            nc.gpsimd.collective_compute(
                "AllGather",
                mybir.AluOpType.bypass,
                replica_groups=[[0, 2, 4, 6], [1, 3, 5, 7]],
                ins=[cc_in[:].opt()],
                outs=[cc_out[:].opt()],
            )
            T4 = spool.tile([4, F], BF16)
            nc.sync.dma_start(T4[:], cc_out[:])

            # --- merged group-sum + transpose: 4 matmuls [4,128]x[4,1] ---
            # psum_t[p, r] = sum_k T4[k, 128r+p]
            psum_t = psum.tile([128, 4], F32, name="psum_t")
            for r in range(4):
                nc.tensor.matmul(
                    psum_t[:, r : r + 1],
                    T4[0:4, 128 * r : 128 * (r + 1)],
                    ones4[0:4, 0:1],
                    start=True, stop=True,
                )
            acc = spool.tile([128, 4], F32)
            nc.vector.tensor_tensor(acc[:], psum_t[:], b0s[:],
                                    op=mybir.AluOpType.add)
            uvr = spool.tile([128, 4], BF16)
            nc.vector.tensor_relu(uvr[:], acc[:])

            # --- layer 2 (bias-seeded) + own-path head ---
            psum2 = psum.tile([1, F], F32, name="psum2")
            nc.tensor.matmul(psum2[:], one_b[0:1, :], b1_sb[:],
                             start=True, stop=False)
            for r in range(4):
                nc.tensor.matmul(
                    psum2[:],
                    uvr[:, r : r + 1],
                    w1_sb[:, r * F : (r + 1) * F],
                    start=False,
                    stop=(r == 3),
                )
            junk = spool.tile([1, F], F32)
            st = spool.tile([1, 1], F32)
            nc.vector.scalar_tensor_tensor(
                junk[:], psum2[:], 0.0, hw_sb[:],
                op0=mybir.AluOpType.max, op1=mybir.AluOpType.mult,
                accum_out=st[:],
            )
            sc2 = spool.tile([1, 1], F32)
            nc.vector.tensor_tensor(sc2[:], st[:], hb_sb[:],
                                    op=mybir.AluOpType.add)
            # scsel = (sc2, 0) on even cores, (0, sc2) on odd cores
            scsel = spool.tile([1, 2], F32)
            nc.vector.tensor_scalar(
                scsel[:], selm[:], sc2[0:1, 0:1], None,
                op0=mybir.AluOpType.mult,
            )
            psum_bc = psum.tile([128, 2], F32, name="psum_bc")
            nc.tensor.matmul(psum_bc[:], ones128[:], scsel[:],
                             start=True, stop=True)
            st_T = spool.tile([128, 2], F32)
            nc.vector.tensor_copy(st_T[:], psum_bc[:])
            vals = spool.tile([128, 128], F32)
            nc.vector.tensor_scalar(
                vals[:], xo[:], st_T[:, 0:1], st_T[:, 1:2],
                op0=mybir.AluOpType.mult, op1=mybir.AluOpType.add,
            )
            nc.sync.dma_start(
                vals_out.ap().rearrange("(p t) -> p t", p=128), vals[:]
            )

    nc.compile()
    return nc


def _quant_w0(W0):
    """Per-column absmax fp8 quantization. Returns (W0q fp8, s per-col)."""
    W0 = np.asarray(W0, dtype=np.float32)
    s = np.abs(W0).max(axis=0).astype(np.float64) / 240.0
    s = np.where(s == 0, 1.0, s)
    W0q = (W0 / s).astype(NP_FP8)
    return W0q, s


def _pack_w0(W0q, q):
    # [65536, 512] -> this quarter's (w0a, w0b, w0c) DoubleRow blocks
    A = W0q.reshape(128, 512, F)[:, 128 * q : 128 * (q + 1), :]
    flat = np.ascontiguousarray(
        A.reshape(128, 64, 2, F).transpose(1, 0, 2, 3)
    ).reshape(64, 128, 2 * F)
    return np.ascontiguousarray(
        flat.reshape(NB, 8, 128, 1024).transpose(0, 2, 1, 3)
    ).reshape(NB, 128, 8192)


def _pack_w1(W1e):
    # w1p[p, r*F+n] = W1e[128r+p, n]
    return np.ascontiguousarray(
        W1e.reshape(4, 128, F).transpose(1, 0, 2)
    ).reshape(128, 4 * F)


def make_in_maps(
    x, u_W0, u_b0, u_W1, u_b1, v_W0, v_b0, v_W1, v_b1,
    us_W, us_b, ut_W, ut_b, even_indices, odd_indices,
):
    x = np.asarray(x, dtype=np.float32)
    xe = x[np.asarray(even_indices)].astype(np.float32)
    xo = x[np.asarray(odd_indices)].astype(np.float32)
    xe_m = xe.reshape(128, 512)

    sx = 240.0 / max(np.abs(xe).max(), 1e-30)
    xe_q = (xe_m * sx).astype(NP_FP8)

    groups = {}
    for gname, W0, b0, W1, b1, hW, hb in [
        ("u", u_W0, u_b0, u_W1, u_b1, us_W, us_b),
        ("v", v_W0, v_b0, v_W1, v_b1, ut_W, ut_b),
    ]:
        W0q, s = _quant_w0(W0)
        se = s / sx
        W1e = np.asarray(W1, np.float64) * se[:, None]
        b0se = (np.asarray(b0, np.float64) / se).astype(np.float32)
        groups[gname] = dict(
            W0q=W0q,
            w1=_pack_w1(W1e.astype(NP_BF16)),
            b0s=np.ascontiguousarray(b0se.reshape(4, 128).T),
            b1=np.asarray(b1, np.float32).astype(NP_BF16)[None, :],
            hw=np.asarray(hW, np.float32)[:, 0][None, :],
            hb=np.asarray(hb, np.float32)[None, :],
        )

    ones4 = np.ones((4, 1), dtype=NP_BF16)
    rpc = VH // 4
    in_maps = []
    for j in range(NCORES):
        gname = "u" if j % 2 == 0 else "v"
        q = j // 2
        gr = groups[gname]
        selm = np.zeros((1, 2), dtype=np.float32)
        selm[0, j % 2] = 1.0
        # xe pair layout: within each 32-col block, cols 0-15 hold the
        # even chunk of pairs dg=16b+i, cols 16-31 the odd chunk
        xe_core = xe_q[:, 128 * q : 128 * (q + 1)]
        xe_dr = np.ascontiguousarray(
            xe_core.reshape(128, 4, 16, 2).transpose(0, 1, 3, 2)
        ).reshape(128, 128)
        in_maps.append(
            {
                "w0b": _pack_w0(gr["W0q"], q),
                "xe8": xe_dr,
                "w1": gr["w1"],
                "b0s": gr["b0s"],
                "b1": gr["b1"],
                "hw": gr["hw"],
                "hb": gr["hb"],
                "selm": selm,
                "ones4": ones4,
                "xo_t": xo[rpc * q : rpc * (q + 1)].reshape(128, 128),
            }
        )
    return in_maps


def kernel(
    x, u_W0, u_b0, u_W1, u_b1, v_W0, v_b0, v_W1, v_b1,
    us_W, us_b, ut_W, ut_b, even_indices, odd_indices,
):
    x = np.asarray(x, dtype=np.float32)
    odd_indices = np.asarray(odd_indices)

    with _lock:
        if "nc" not in _cache:
            _cache["nc"] = build_nc()
    nc = _cache["nc"]

    in_maps = make_in_maps(
        x, u_W0, u_b0, u_W1, u_b1, v_W0, v_b0, v_W1, v_b1,
        us_W, us_b, ut_W, ut_b, even_indices, odd_indices,
    )

    res = run_bass_kernel_spmd(nc, in_maps, core_ids=list(range(NCORES)))
    # host combine: vals[pair p] = u_s*xo[pair] (even core) + u_t (odd core)
    vals = np.concatenate([
        res.results[2 * p]["vals_out"] + res.results[2 * p + 1]["vals_out"]
        for p in range(4)
    ])

    imag = np.zeros(V, dtype=np.float32)
    np.add.at(imag, odd_indices, vals)
    y = np.empty(V, dtype=np.complex64)
    y.real = x
    y.imag = imag
    return y


# revision 5
# speedup vs baseline: 1.2672x; 1.2672x over previous
"""Trainium2 Bass kernel v6 for nn_AffineContour — parity-split AllGather.

Design (8 cores):
- Even cores compute the u-MLP, odd cores the v-MLP; each parity group
  of 4 row-shards its (65536, 512) W0 4 ways (8.39 MB fp8 per core).
- W0 host-quantized to fp8e4 per-column; x_even fp8 with one global
  scale; scales fold into W1 rows (relu(s z) = s relu(z), s>0); W1 bf16.
- GEMV runs in fp8 DoubleRow perf mode: 64 matmuls contracting 256 rows
  each (the existing weight packing is already the DoubleRow interleave).
- ONE ncfw AllGather with two parity replica groups [[0,2,4,6],[1,3,5,7]]
  exchanges the [1,512] bf16 partials inside each parity only (4-rank
  mesh). Summing + transposing the 4 gathered rows is 4 small matmuls.
- The parities never talk on-device: vals = u_s*x_odd + u_t splits into
  a u-only and a v-only term; even core 2p outputs u_s*xo[pair p], odd
  core 2p+1 outputs u_t broadcast, and the HOST adds the two vectors.
"""

import threading

import ml_dtypes
import numpy as np

import concourse.bacc as bacc
import concourse.mybir as mybir
import concourse.tile as tile
from concourse.bass_utils import run_bass_kernel_spmd

V = 131072
VH = 65536
F = 512
NCORES = 8
CHUNKS = 128          # 128-row chunks per core (16384 rows)
DG = 64               # DoubleRow double-groups (2 chunks each)
NB = 8                # weight DMA blocks (1 MB each)
F32 = mybir.dt.float32
BF16 = mybir.dt.bfloat16
FP8 = mybir.dt.float8e4

NP_FP8 = ml_dtypes.float8_e4m3   # IEEE e4m3, max 240 — matches TRN fp8e4
NP_BF16 = ml_dtypes.bfloat16

_lock = threading.Lock()
_cache = {}


def build_nc():
    nc = bacc.Bacc(
        "TRN2",
        debug=False,
        enable_asserts=False,
        target_bir_lowering=False,
        num_devices=NCORES,
    )

    w0b = nc.dram_tensor("w0b", [NB, 128, 8192], FP8, kind="ExternalInput")
    xe8_d = nc.dram_tensor("xe8", [128, CHUNKS], FP8, kind="ExternalInput")
    w1_d = nc.dram_tensor("w1", [128, 4 * F], BF16, kind="ExternalInput")
    b0s_d = nc.dram_tensor("b0s", [128, 4], F32, kind="ExternalInput")
    b1_d = nc.dram_tensor("b1", [1, F], BF16, kind="ExternalInput")
    hw_d = nc.dram_tensor("hw", [1, F], F32, kind="ExternalInput")
    hb_d = nc.dram_tensor("hb", [1, 1], F32, kind="ExternalInput")
    selm_d = nc.dram_tensor("selm", [1, 2], F32, kind="ExternalInput")
    ones4_d = nc.dram_tensor("ones4", [4, 1], BF16, kind="ExternalInput")
    xo_d = nc.dram_tensor("xo_t", [128, 128], F32, kind="ExternalInput")
    vals_out = nc.dram_tensor("vals_out", [VH // 4], F32,
                              kind="ExternalOutput")

    with tile.TileContext(nc) as tc:
        with (
            tc.tile_pool(name="wpool", bufs=6) as wpool,
            tc.tile_pool(name="spool", bufs=1) as spool,
            tc.tile_pool(name="psum", bufs=1, space="PSUM") as psum,
            tc.tile_pool(name="dram", bufs=1, space="DRAM") as dram,
        ):
            xe = spool.tile([128, CHUNKS], FP8)
            nc.sync.dma_start(xe[:], xe8_d[:])
            w1_sb = spool.tile([128, 4 * F], BF16)
            b0s = spool.tile([128, 4], F32)
            nc.gpsimd.dma_start(b0s[:], b0s_d[:])
            b1_sb = spool.tile([1, F], BF16)
            nc.gpsimd.dma_start(b1_sb[:], b1_d[:])
            hw_sb = spool.tile([1, F], F32)
            nc.gpsimd.dma_start(hw_sb[:], hw_d[:])
            hb_sb = spool.tile([1, 1], F32)
            nc.gpsimd.dma_start(hb_sb[:], hb_d[:])
            selm = spool.tile([1, 2], F32)
            nc.gpsimd.dma_start(selm[:], selm_d[:])
            ones4 = spool.tile([4, 1], BF16)
            nc.gpsimd.dma_start(ones4[:], ones4_d[:])
            xo = spool.tile([128, 128], F32)
            nc.gpsimd.dma_start(xo[:], xo_d[:])

            one_b = spool.tile([1, 1], BF16)
            nc.vector.memset(one_b[:], 1.0)
            ones128 = spool.tile([1, 128], F32)
            nc.vector.memset(ones128[:], 1.0)

            # --- layer-1 GEMV: 64 DoubleRow matmuls, K=256 each ---
            # Tile tracks DMA deps per TILE, so block size sets how long
            # the first/last matmul waits: stream 4x128KB head pieces,
            # 7x1MB middle blocks, 4x128KB tail pieces. The 512 KB w1
            # load is deferred behind most of the weight stream.
            dma_engines = [nc.sync, nc.scalar]
            psum1 = psum.tile([1, F], F32, name="psum1")

            def dr_matmul(dg, wt, t):
                # xe pair for dg lives at cols 32b+i and 32b+i+16
                # (LDWEIGHTS DoubleRow needs 16-element pair stride)
                b, i = dg // 16, dg % 16
                nc.tensor.matmul(
                    psum1[:],
                    xe[:, 32 * b : 32 * b + 32].rearrange(
                        "p (k i) -> p k i", k=2)[:, :, i : i + 1],
                    wt[:, 1024 * t : 1024 * (t + 1)].rearrange(
                        "p (k n) -> p k n", k=2),
                    start=(dg == 0),
                    stop=(dg == DG - 1),
                    perf_mode=mybir.MatmulPerfMode.DoubleRow,
                )

            for g in range(NB):
                wt = wpool.tile([128, 8192], FP8, tag="wmid")
                if g == NB - 1:
                    for q in range(4):
                        dma_engines[q % 2].dma_start(
                            wt[:, 2048 * q : 2048 * (q + 1)],
                            w0b.ap()[g][:, 2048 * q : 2048 * (q + 1)],
                        )
                else:
                    dma_engines[g % 2].dma_start(wt[:], w0b.ap()[g])
                if g == 6:
                    nc.scalar.dma_start(w1_sb[:], w1_d[:])
                for t in range(8):
                    dr_matmul(8 * g + t, wt, t)
            partial = spool.tile([1, F], BF16)
            nc.vector.tensor_copy(partial[:], psum1[:])

            # --- AllGather [1,512] bf16 within each parity group of 4 ---
            cc_in = dram.tile([1, F], BF16)
            cc_out = dram.tile([4, F], BF16)
            nc.sync.dma_start(cc_in[:], partial[:])
            nc.gpsimd.collective_compute(
                "AllGather",
                mybir.AluOpType.bypass,
                replica_groups=[[0, 2, 4, 6], [1, 3, 5, 7]],
                ins=[cc_in[:].opt()],
                outs=[cc_out[:].opt()],
            )
            T4 = spool.tile([4, F], BF16)
            nc.sync.dma_start(T4[:], cc_out[:])

            # --- merged group-sum + transpose: 4 matmuls [4,128]x[4,1] ---
            # psum_t[p, r] = sum_k T4[k, 128r+p]
            psum_t = psum.tile([128, 4], F32, name="psum_t")
            for r in range(4):
                nc.tensor.matmul(
                    psum_t[:, r : r + 1],
                    T4[0:4, 128 * r : 128 * (r + 1)],
                    ones4[0:4, 0:1],
                    start=True, stop=True,
                )
            acc = spool.tile([128, 4], F32)
            nc.vector.tensor_tensor(acc[:], psum_t[:], b0s[:],
                                    op=mybir.AluOpType.add)
            uvr = spool.tile([128, 4], BF16)
            nc.vector.tensor_relu(uvr[:], acc[:])

            # --- layer 2 (bias-seeded) + own-path head ---
            psum2 = psum.tile([1, F], F32, name="psum2")
            nc.tensor.matmul(psum2[:], one_b[0:1, :], b1_sb[:],
                             start=True, stop=False)
            for r in range(4):
                nc.tensor.matmul(
                    psum2[:],
                    uvr[:, r : r + 1],
                    w1_sb[:, r * F : (r + 1) * F],
                    start=False,
                    stop=(r == 3),
                )
            junk = spool.tile([1, F], F32)
            st = spool.tile([1, 1], F32)
            nc.vector.scalar_tensor_tensor(
                junk[:], psum2[:], 0.0, hw_sb[:],
                op0=mybir.AluOpType.max, op1=mybir.AluOpType.mult,
                accum_out=st[:],
            )
            sc2 = spool.tile([1, 1], F32)
            nc.vector.tensor_tensor(sc2[:], st[:], hb_sb[:],
                                    op=mybir.AluOpType.add)
            # scsel = (sc2, 0) on even cores, (0, sc2) on odd cores
            scsel = spool.tile([1, 2], F32)
            nc.vector.tensor_scalar(
                scsel[:], selm[:], sc2[0:1, 0:1], None,
                op0=mybir.AluOpType.mult,
            )
            psum_bc = psum.tile([128, 2], F32, name="psum_bc")
            nc.tensor.matmul(psum_bc[:], ones128[:], scsel[:],
                             start=True, stop=True)
            st_T = spool.tile([128, 2], F32)
            nc.vector.tensor_copy(st_T[:], psum_bc[:])
            vals = spool.tile([128, 128], F32)
            nc.vector.tensor_scalar(
                vals[:], xo[:], st_T[:, 0:1], st_T[:, 1:2],
                op0=mybir.AluOpType.mult, op1=mybir.AluOpType.add,
            )
            nc.sync.dma_start(
                vals_out.ap().rearrange("(p t) -> p t", p=128), vals[:]
            )

    nc.compile()
    return nc


def _quant_w0(W0):
    """Per-column absmax fp8 quantization. Returns (W0q fp8, s per-col)."""
    W0 = np.asarray(W0, dtype=np.float32)
    s = np.abs(W0).max(axis=0).astype(np.float64) / 240.0
    s = np.where(s == 0, 1.0, s)
    W0q = (W0 / s).astype(NP_FP8)
    return W0q, s


def _pack_w0(W0q, q):
    # [65536, 512] -> this quarter's (w0a, w0b, w0c) DoubleRow blocks
    A = W0q.reshape(128, 512, F)[:, 128 * q : 128 * (q + 1), :]
    flat = np.ascontiguousarray(
        A.reshape(128, 64, 2, F).transpose(1, 0, 2, 3)
    ).reshape(64, 128, 2 * F)
    return np.ascontiguousarray(
        flat.reshape(NB, 8, 128, 1024).transpose(0, 2, 1, 3)
    ).reshape(NB, 128, 8192)


def _pack_w1(W1e):
    # w1p[p, r*F+n] = W1e[128r+p, n]
    return np.ascontiguousarray(
        W1e.reshape(4, 128, F).transpose(1, 0, 2)
    ).reshape(128, 4 * F)


def make_in_maps(
    x, u_W0, u_b0, u_W1, u_b1, v_W0, v_b0, v_W1, v_b1,
    us_W, us_b, ut_W, ut_b, even_indices, odd_indices,
):
    x = np.asarray(x, dtype=np.float32)
    xe = x[np.asarray(even_indices)].astype(np.float32)
    xo = x[np.asarray(odd_indices)].astype(np.float32)
    xe_m = xe.reshape(128, 512)

    sx = 240.0 / max(np.abs(xe).max(), 1e-30)
    xe_q = (xe_m * sx).astype(NP_FP8)

    groups = {}
    for gname, W0, b0, W1, b1, hW, hb in [
        ("u", u_W0, u_b0, u_W1, u_b1, us_W, us_b),
        ("v", v_W0, v_b0, v_W1, v_b1, ut_W, ut_b),
    ]:
        W0q, s = _quant_w0(W0)
        se = s / sx
        W1e = np.asarray(W1, np.float64) * se[:, None]
        b0se = (np.asarray(b0, np.float64) / se).astype(np.float32)
        groups[gname] = dict(
            W0q=W0q,
            w1=_pack_w1(W1e.astype(NP_BF16)),
            b0s=np.ascontiguousarray(b0se.reshape(4, 128).T),
            b1=np.asarray(b1, np.float32).astype(NP_BF16)[None, :],
            hw=np.asarray(hW, np.float32)[:, 0][None, :],
            hb=np.asarray(hb, np.float32)[None, :],
        )

    ones4 = np.ones((4, 1), dtype=NP_BF16)
    rpc = VH // 4
    in_maps = []
    for j in range(NCORES):
        gname = "u" if j % 2 == 0 else "v"
        q = j // 2
        gr = groups[gname]
        selm = np.zeros((1, 2), dtype=np.float32)
        selm[0, j % 2] = 1.0
        # xe pair layout: within each 32-col block, cols 0-15 hold the
        # even chunk of pairs dg=16b+i, cols 16-31 the odd chunk
        xe_core = xe_q[:, 128 * q : 128 * (q + 1)]
        xe_dr = np.ascontiguousarray(
            xe_core.reshape(128, 4, 16, 2).transpose(0, 1, 3, 2)
        ).reshape(128, 128)
        in_maps.append(
            {
                "w0b": _pack_w0(gr["W0q"], q),
                "xe8": xe_dr,
                "w1": gr["w1"],
                "b0s": gr["b0s"],
                "b1": gr["b1"],
                "hw": gr["hw"],
                "hb": gr["hb"],
                "selm": selm,
                "ones4": ones4,
                "xo_t": xo[rpc * q : rpc * (q + 1)].reshape(128, 128),
            }
        )
    return in_maps


def kernel(
    x, u_W0, u_b0, u_W1, u_b1, v_W0, v_b0, v_W1, v_b1,
    us_W, us_b, ut_W, ut_b, even_indices, odd_indices,
):
    x = np.asarray(x, dtype=np.float32)
    odd_indices = np.asarray(odd_indices)

    with _lock:
        if "nc" not in _cache:
            _cache["nc"] = build_nc()
    nc = _cache["nc"]

    in_maps = make_in_maps(
        x, u_W0, u_b0, u_W1, u_b1, v_W0, v_b0, v_W1, v_b1,
        us_W, us_b, ut_W, ut_b, even_indices, odd_indices,
    )

    res = run_bass_kernel_spmd(nc, in_maps, core_ids=list(range(NCORES)))
    # host combine: vals[pair p] = u_s*xo[pair] (even core) + u_t (odd core)
    vals = np.concatenate([
        res.results[2 * p]["vals_out"] + res.results[2 * p + 1]["vals_out"]
        for p in range(4)
    ])

    imag = np.zeros(V, dtype=np.float32)
    np.add.at(imag, odd_indices, vals)
    y = np.empty(V, dtype=np.complex64)
    y.real = x
    y.imag = imag
    return y
